# revision 81
# baseline (speedup 1.0000x reference)
"""Trainium2 Bass kernel for AlphaFold-style gated attention.

Reference math (B=4, N=1024, C=512, H=8, CH=64):
    q = (q_x @ Wq) / 8 ; k = kv_x @ Wk ; v = kv_x @ Wv
    s = q k^T + bias_mask[b,k] + bias_pair[h,q,k]
    a = softmax_k(s) ; o = a @ v
    g = sigmoid(q_x @ Wg + bg)
    out = (o*g) @ Wo + bo

Sharding: 8 cores = (batch b in 0..3) x (q-half qh in 0..1). Zero collectives.

v2 design notes:
  - q/k/g projections run in fp8e4 with DoubleRow perf mode (2
    contraction tiles per instruction, 0.5 cycles/row). Host scales
    weights (x8/x8/x8 after inv_sqrt) into fp8 range; the exp/sigmoid
    activations undo the scale via their constant `scale` operand.
    Scores stay bf16 so q/k carry only one fp8 quantization layer
    (rel err ~1.6e-2 vs the 2e-2 gate).
  - bias_mask folded as the exp() per-partition bias (mask[k] with k on
    partitions) - no em-scaled v needed; the softmax denominator comes
    from a constant-1.0 65th v column.
  - exp(bias_pair) multiplies exp(scores) on DVE/Pool (bf16, 2x mode).
  - AV runs q-on-partitions: one [128q, 65] 8-matmul chain per
    (q-chunk, head) - free dim 65 instead of 512 halves the AV cost -
    and normalization becomes a per-partition reciprocal +
    tensor_scalar multiply, killing the 1/d broadcast matmuls of v1.
  - o/d transposed back to [feat, q] via 16 PE transposes; the gate
    multiply is fused into the transpose's PSUM->SBUF copy.
  - Schedule: the ACT exp stream (32x ~1us) is the pacer. Scores
    trickle through 2-3 PSUM slots at exp pace; projections and the
    previous pair's AV groups are sliced into the stall windows at kc
    granularity. Pair tiles triple-buffer so all four DMA back-to-back;
    mults for pairs 2/3 ride inline behind their exps. The out
    projection opens accumulation chains early (ci 0..2 pre-run), and
    the four output stores dispatch from alternating sequencers.
"""

import sys

import numpy as np

if "/opt/trn_rl_repo" not in sys.path:
    sys.path.insert(0, "/opt/trn_rl_repo")

import ml_dtypes

import concourse.bass as bass
import concourse.tile as tile
from concourse import bacc, mybir
from concourse.bass_utils import run_bass_kernel_spmd

B, N, C, H, CH = 4, 1024, 512, 8, 64
R = 512          # q rows per core
KC = N // 128    # 8 k chunks of 128
CC = C // 128    # 4 feature chunks of 128
F32 = mybir.dt.float32
BF16 = mybir.dt.bfloat16
FP8 = mybir.dt.float8e4
BF = ml_dtypes.bfloat16
F8 = ml_dtypes.float8_e4m3fn

SQ = 64.0        # wq8 = Wq * inv_sqrt * SQ
SK = 8.0         # wk8 = Wk * SK
SG = 8.0         # wg8 = Wg * SG
SEXP = 1.0 / (SQ * SK)


def build(finalize=True):
    nc = bacc.Bacc("TRN2", target_bir_lowering=False, debug=False)

    qx8 = nc.dram_tensor("qx8", [C, R], FP8, kind="ExternalInput").ap()
    bgr = nc.dram_tensor("bgr", [128, CC], F32, kind="ExternalInput").ap()
    kv8 = nc.dram_tensor("kv8", [C, N], FP8, kind="ExternalInput").ap()
    kvt = nc.dram_tensor("kvt", [C, N], BF16, kind="ExternalInput").ap()
    wq8 = nc.dram_tensor("wq8", [C, C], FP8, kind="ExternalInput").ap()
    wk8 = nc.dram_tensor("wk8", [C, C], FP8, kind="ExternalInput").ap()
    wg8 = nc.dram_tensor("wg8", [C, C], FP8, kind="ExternalInput").ap()
    wv = nc.dram_tensor("wv", [C, C], BF16, kind="ExternalInput").ap()
    wo = nc.dram_tensor("wo", [C, C], BF16, kind="ExternalInput").ap()
    pairt = nc.dram_tensor("pairt", [H, N, R], BF16, kind="ExternalInput").ap()
    maskb = nc.dram_tensor("maskb", [128, KC], F32, kind="ExternalInput").ap()
    bor = nc.dram_tensor("bor", [128, CC], F32, kind="ExternalInput").ap()
    ident = nc.dram_tensor("ident", [128, 128], BF16, kind="ExternalInput").ap()
    out = nc.dram_tensor("out", [C, R], BF16, kind="ExternalOutput").ap()

    with tile.TileContext(nc) as tc:
        _body(tc, qx8, kv8, kvt, wq8, wk8, wg8, wv, wo, pairt, maskb, bor,
              bgr, ident, out)
    if finalize:
        nc.finalize()
    return nc


def _body(tc, qx8, kv8, kvt, wq8, wk8, wg8, wv, wo, pairt, maskb, bor,
          bgr, ident, out):
    nc = tc.nc
    Exp = mybir.ActivationFunctionType.Exp
    Sigmoid = mybir.ActivationFunctionType.Sigmoid
    Ident = mybir.ActivationFunctionType.Identity

    with (
        tc.tile_pool(name="keep", bufs=1) as keep,
        tc.tile_pool(name="sb", bufs=8) as sb,
        tc.tile_pool(name="pairp", bufs=3) as pairp,
        tc.tile_pool(name="dp", bufs=2) as dp,
        tc.tile_pool(name="psA", bufs=2, space="PSUM") as psA,
        tc.tile_pool(name="psS", bufs=2, space="PSUM") as psS,
        tc.tile_pool(name="psO", bufs=1, space="PSUM") as psO,
    ):
        # ---- static tiles ----
        maskb_sb = keep.tile([128, KC], F32, tag="maskb")
        bor_sb = keep.tile([128, CC], F32, tag="bor")
        bgr_sb = keep.tile([128, CC], F32, tag="bgr")
        ident_sb = keep.tile([128, 128], BF16, tag="ident")
        qx8_sb = keep.tile([128, CC, R], FP8, tag="qx8")
        kv8_sb = keep.tile([128, CC, N], FP8, tag="kv8")
        kvt_sb = keep.tile([128, CC, N], BF16, tag="kvt")
        w8_sb = {}
        for wname in ("wq8", "wk8", "wg8"):
            w8_sb[wname] = keep.tile([128, CC, C], FP8, tag=wname,
                                     name=f"w_{wname}")
        wv_sb = keep.tile([128, CC, C], BF16, tag="wv")
        wo_sb = keep.tile([128, CC, C], BF16, tag="wo")

        # ---- input DMAs, ordered by first consumer (kproj inputs lead;
        # the small tiles ride later so their HWDGE setup time doesn't
        # delay the first projection) ----
        nc.sync.dma_start(out=kv8_sb, in_=kv8.rearrange("(cc p) n -> p cc n", p=128))
        nc.sync.dma_start(out=w8_sb["wk8"], in_=wk8.rearrange("(cc p) o -> p cc o", p=128))
        nc.sync.dma_start(out=qx8_sb, in_=qx8.rearrange("(cc p) r -> p cc r", p=128))
        nc.sync.dma_start(out=w8_sb["wq8"], in_=wq8.rearrange("(cc p) o -> p cc o", p=128))
        nc.sync.dma_start(out=maskb_sb, in_=maskb)
        nc.sync.dma_start(out=kvt_sb, in_=kvt.rearrange("(cc p) n -> p cc n", p=128))
        nc.sync.dma_start(out=wv_sb, in_=wv.rearrange("(cc p) o -> p cc o", p=128))
        nc.sync.dma_start(out=w8_sb["wg8"], in_=wg8.rearrange("(cc p) o -> p cc o", p=128))
        nc.sync.dma_start(out=bgr_sb, in_=bgr)
        nc.sync.dma_start(out=ident_sb, in_=ident)
        nc.sync.dma_start(out=bor_sb, in_=bor)

        pair_tiles = {}

        def fetch_pair(hp):
            h0 = 2 * hp
            t = pairp.tile([128, KC, 2, R], BF16, tag="pair",
                           name=f"pairt_{hp}")
            for hi in range(2):
                nc.sync.dma_start(
                    out=t[:, :, hi, :],
                    in_=pairt[h0 + hi].rearrange("(kc p) r -> p kc r", p=128))
            pair_tiles[hp] = t

        fetch_pair(0)
        fetch_pair(1)

        # ---- activation / projection result tiles ----
        # qT/kT bf16 [feat, q/n], head pair hp on partition halves 0:64 /
        # 64:128 of feature chunk hp (v1 layout; scores stay bf16 so q/k
        # carry only the projection-input fp8 quantization, not a second
        # storage quantization - keeps rel err at ~1.3e-2).
        qT = keep.tile([128, CC, R], BF16, tag="qT")
        kT = keep.tile([128, CC, N], BF16, tag="kT")
        gT = keep.tile([128, CC, R], BF16, tag="gT")   # [feat, q] gate
        vS = keep.tile([128, KC, H, 65], BF16, tag="vS")
        xg_q = keep.tile([128, 4, C], BF16, tag="xg_q")   # [q(qc), feat]
        xgT = keep.tile([128, CC, R], BF16, tag="xgT")    # [feat, q]

        # ones column for the softmax denominator: vS[:, kc, h, 64] = 1
        nc.vector.memset(vS[:, :, :, 64:65], 1.0)

        # ---- projection emitters (fp8 DoubleRow for k/q/g, bf16 for v) ----
        def emit_kproj(ci, nh):
            ps = psA.tile([128, R], F32, tag="ps", name=f"k_{ci}_{nh}")
            for c2 in range(2):
                nc.tensor.matmul(
                    ps,
                    w8_sb["wk8"][:, 2 * c2:2 * c2 + 2,
                                 ci * 128:(ci + 1) * 128],
                    kv8_sb[:, 2 * c2:2 * c2 + 2, nh * 512:(nh + 1) * 512],
                    start=(c2 == 0), stop=(c2 == 1),
                    perf_mode=mybir.MatmulPerfMode.DoubleRow)
            nc.vector.tensor_copy(kT[:, ci, nh * 512:(nh + 1) * 512], ps)

        def emit_qproj(ci, on_act):
            ps = psA.tile([128, R], F32, tag="ps", name=f"q_{ci}")
            for c2 in range(2):
                nc.tensor.matmul(
                    ps,
                    w8_sb["wq8"][:, 2 * c2:2 * c2 + 2,
                                 ci * 128:(ci + 1) * 128],
                    qx8_sb[:, 2 * c2:2 * c2 + 2, :],
                    start=(c2 == 0), stop=(c2 == 1),
                    perf_mode=mybir.MatmulPerfMode.DoubleRow)
            if on_act:
                nc.scalar.copy(qT[:, ci, :], ps)
            else:
                nc.vector.tensor_copy(qT[:, ci, :], ps)

        g_ps = {}

        def emit_gproj_mm(cc):
            ps = psA.tile([128, R], F32, tag="ps", name=f"g_{cc}")
            for c2 in range(2):
                nc.tensor.matmul(
                    ps,
                    w8_sb["wg8"][:, 2 * c2:2 * c2 + 2,
                                 cc * 128:(cc + 1) * 128],
                    qx8_sb[:, 2 * c2:2 * c2 + 2, :],
                    start=(c2 == 0), stop=(c2 == 1),
                    perf_mode=mybir.MatmulPerfMode.DoubleRow)
            g_ps[cc] = ps

        def emit_sigmoid(cc):
            # g = sigmoid(psum/SG + bg); emitted into the ACT stream at a
            # point where it doesn't delay the exp chain start.
            nc.scalar.activation(gT[:, cc, :], g_ps.pop(cc), Sigmoid,
                                 bias=bgr_sb[:, cc:cc + 1], scale=1.0 / SG)

        def emit_vproj(kc):
            ps = psA.tile([128, R], F32, tag="ps", name=f"v_{kc}")
            for ci in range(CC):
                nc.tensor.matmul(
                    ps, kvt_sb[:, ci, kc * 128:(kc + 1) * 128],
                    wv_sb[:, ci, :], start=(ci == 0), stop=(ci == CC - 1))
            nc.vector.tensor_copy(vS[:, kc, :, 0:64], ps)

        # ---- attention over head pairs ----
        # scores(hp) -> exp -> pair-mult stream one kc at a time (a_t
        # buffered for the whole hp); AV then runs one (qc, head) group at
        # a time, each an 8-matmul contraction chain into its own PSUM
        # bank ([128q, 65] padded - one accumulation group per 2KB zero
        # region). Normalize per-group with a reciprocal + tensor-scalar
        # multiply, then transpose that head-pair's x columns with the
        # gate multiply fused into the PSUM->SBUF copy.

        def emit_score_exp(hp, kc):
            # hp0 borrows the (idle until hp1) AV pool as a third score
            # slot so its scores drain ahead of the sliced projections
            if hp == 0 and kc % 3 == 2:
                st = psO.tile([128, 2, R], F32, tag="ov",
                              name=f"st_{hp}_{kc}")
            else:
                st = psS.tile([128, 2, R], F32, tag="st")
            for hi in range(2):
                p0 = 64 * hi
                nc.tensor.matmul(
                    st[:, hi, :],
                    kT[p0:p0 + 64, hp, kc * 128:(kc + 1) * 128],
                    qT[p0:p0 + 64, hp, :],
                    start=True, stop=True)
            # e deep-buffered so the exp stream never waits on the
            # pair-mults (which chase the pairt DMA)
            e = sb.tile([128, 2, R], BF16, tag="e", bufs=16,
                        name=f"e_{hp}_{kc}")
            nc.scalar.activation(e, st, Exp, bias=maskb_sb[:, kc:kc + 1],
                                 scale=SEXP)
            return e

        def emit_mults(hp, es, pool_kcs=(2, 5)):
            pairt_h = pair_tiles[hp]
            ats = []
            for kc in range(KC):
                a_t = sb.tile([128, 2, R], BF16, tag="at", bufs=18,
                              name=f"a_{hp}_{kc}")
                eng = nc.gpsimd if kc in pool_kcs else nc.vector
                eng.tensor_mul(a_t, es[kc], pairt_h[:, kc, :, :])
                ats.append(a_t)
            return ats

        def emit_av_group(hp, ats, qc, last_hp=False):
            # one [128q, 65] accumulation chain per (qc, head); the two
            # heads of the pair share a 2-bank tile (one group per bank)
            # so the reciprocal batches over both denominator columns.
            # The last pair alternates with the score pool's freed banks
            # so its AV can triple-buffer at the tail.
            h0 = 2 * hp
            if last_hp and qc % 2 == 0:
                ov = psS.tile([128, 2, 65], F32, tag="st",
                              padded_shape=[128, 2, 512],
                              name=f"ov_{hp}_{qc}")
            else:
                ov = psO.tile([128, 2, 65], F32, tag="ov",
                              padded_shape=[128, 2, 512],
                              name=f"ov_{hp}_{qc}")
            for hi in range(2):
                for kc in range(KC):
                    nc.tensor.matmul(
                        ov[:, hi, :],
                        ats[kc][:, hi, qc * 128:(qc + 1) * 128],
                        vS[:, kc, h0 + hi, :],
                        start=(kc == 0), stop=(kc == KC - 1))
            dv = dp.tile([128, 2], F32, tag="dinv",
                         name=f"dinv_{hp}_{qc}")
            nc.vector.reciprocal(dv, ov[:, :, 64])
            for hi in range(2):
                if last_hp and hi == 1:
                    # ACT is idle after the last exp; let it normalize
                    # one head of each pair to shorten the tail
                    nc.scalar.mul(
                        xg_q[:, qc, (h0 + hi) * 64:(h0 + hi + 1) * 64],
                        ov[:, hi, 0:64], dv[:, hi:hi + 1])
                else:
                    nc.vector.tensor_scalar_mul(
                        xg_q[:, qc, (h0 + hi) * 64:(h0 + hi + 1) * 64],
                        ov[:, hi, 0:64], dv[:, hi:hi + 1])

        def emit_transpose(fc, qc):
            # x columns of head pair fc are final; bring them to [feat, q]
            # with the gate fused into the PSUM->SBUF copy. fc3 borrows
            # score-pool slots (psA is held by open out-proj chains then).
            if fc == CC - 1:
                pt = psS.tile([128, 128], BF16, tag="st",
                              padded_shape=[128, 2048],
                              name=f"t_{qc}_{fc}")
            else:
                pt = psA.tile([128, 128], BF16, tag="ps",
                              padded_shape=[128, 1024],
                              name=f"t_{qc}_{fc}")
            nc.tensor.transpose(
                pt, xg_q[:, qc, fc * 128:(fc + 1) * 128], ident_sb)
            nc.vector.tensor_mul(
                xgT[:, fc, qc * 128:(qc + 1) * 128], pt,
                gT[:, fc, qc * 128:(qc + 1) * 128])

        def emit_transposes(fc):
            for qc in range(4):
                emit_transpose(fc, qc)

        # ---- out projection: 4 chains, ci 0..2 pre-run during attention ----
        osb_all = keep.tile([128, CC, R], BF16, tag="osb")
        out_r = out.rearrange("(cc p) r -> cc p r", p=128)
        o_ps = {}

        def emit_oproj_partial(cc, pool, tag, ci_hi):
            if cc not in o_ps:
                if tag == "ps":
                    o_ps[cc] = pool.tile([128, R], F32, tag=tag,
                                         name=f"o_{cc}")
                else:
                    o_ps[cc] = pool.tile([128, R], F32, tag=tag,
                                         padded_shape=[128, 1024],
                                         name=f"o_{cc}")
            ps = o_ps[cc]
            for ci in range(ci_hi[0], ci_hi[1]):
                nc.tensor.matmul(
                    ps, wo_sb[:, ci, cc * 128:(cc + 1) * 128],
                    xgT[:, ci, :], start=(ci == 0), stop=(ci == CC - 1))

        def emit_oproj_finish(cc):
            if cc % 2 == 1:
                nc.scalar.activation(osb_all[:, cc, :], o_ps[cc], Ident,
                                     bias=bor_sb[:, cc:cc + 1])
            else:
                nc.vector.tensor_scalar_add(osb_all[:, cc, :], o_ps[cc],
                                            bor_sb[:, cc:cc + 1])
            # dispatch each chunk's store from its own sequencer so the
            # DGE latencies overlap instead of serializing on one queue
            eng = (nc.sync, nc.scalar, nc.sync, nc.scalar)[cc]
            eng.dma_start(out=out_r[cc], in_=osb_all[:, cc, :])

        # ---- schedule ----
        # Scores trickle through the two st slots at exp pace, so the PE
        # queue behind each score matmul gets a "free" stall window; the
        # remaining projections (iteration 0) and the previous pair's AV
        # groups (iterations 1..3) are sliced into those windows at kc
        # granularity to keep every engine's in-order queue flowing.
        emit_kproj(0, 0)
        emit_kproj(0, 1)
        emit_qproj(0, on_act=False)

        def emit_mult(hp, kc, pool_kcs):
            pairt_h = pair_tiles[hp]
            a_t = sb.tile([128, 2, R], BF16, tag="at", bufs=18,
                          name=f"a_{hp}_{kc}")
            eng = nc.gpsimd if kc in pool_kcs else nc.vector
            eng.tensor_mul(a_t, es[kc], pairt_h[:, kc, :, :])
            return a_t

        ats = None
        for hp in range(H // 2):
            es = []
            cur_ats = []
            # pairs 2/3 are resident before their window starts, so their
            # mults ride inline right behind each exp; pairs 0/1 arrive
            # mid-window and must not stall the DVE queue (batched below)
            inline = hp >= 2
            pool_kcs = (2, 5) if hp < 3 else (2,)
            for kc in range(KC):
                es.append(emit_score_exp(hp, kc))
                if inline:
                    cur_ats.append(emit_mult(hp, kc, pool_kcs))
                if hp == 0:
                    # slice the remaining projections into hp0's windows;
                    # only v0..v3 ride between scores (so kc6/kc7 scores
                    # aren't stuck behind them) - the rest follow kc7.
                    if kc < 3:
                        emit_kproj(kc + 1, 0)
                        emit_kproj(kc + 1, 1)
                        emit_qproj(kc + 1, on_act=False)
                    elif kc < 7:
                        lo, hi = {3: (0, 1), 4: (1, 3),
                                  5: (3, 4), 6: (4, 4)}[kc]
                        for v in range(lo, hi):
                            emit_vproj(v)
                    else:
                        for cc in range(CC):
                            emit_gproj_mm(cc)
                        for v in range(4, 8):
                            emit_vproj(v)
                else:
                    if hp == 1 and kc == 0:
                        # sigmoids here: the block fills the ACT stream
                        # while hp1's scores catch up
                        for cc in range(CC):
                            emit_sigmoid(cc)
                    if kc % 2 == 1:
                        emit_av_group(hp - 1, ats, (kc - 1) // 2)
            if hp > 0:
                emit_transposes(hp - 1)
            if hp + 2 <= 3:
                fetch_pair(hp + 2)
            if hp == 1:
                nc.sync.dma_start(
                    out=wo_sb, in_=wo.rearrange("(cc p) o -> p cc o", p=128))
            ats = cur_ats if inline else emit_mults(hp, es, pool_kcs)

        # tail: last pair's AV, per-qc transposes, out projection. The
        # first AV group goes before the out-proj partials so its chain
        # steps chase the pair-mult stream; transposes/gates follow each
        # group so the split out-proj sub-chains can chase them.
        emit_av_group(3, ats, 1, last_hp=True)
        emit_oproj_partial(0, psA, "ps", (0, 3))
        emit_oproj_partial(1, psA, "ps", (0, 3))
        for qc in (0, 2, 3):
            emit_av_group(3, ats, qc, last_hp=True)
        emit_transposes(3)

        for cc in range(2):
            emit_oproj_partial(cc, None, None, (3, 4))
            emit_oproj_finish(cc)
        emit_oproj_partial(2, psO, "ov", (0, 4))
        emit_oproj_finish(2)
        emit_oproj_partial(3, psS, "st", (0, 4))
        emit_oproj_finish(3)


def prep_in_maps(q_x, kv_x, bias_mask, bias_pair, Wq, Wk, Wv, Wg, bg, Wo, bo):
    f32 = np.float32
    inv = 1.0 / np.sqrt(np.float32(CH))

    def to8(x):
        return np.ascontiguousarray(np.asarray(x, f32).astype(F8))

    shared = {
        "wq8": to8(np.asarray(Wq, f32) * (inv * SQ)),
        "wk8": to8(np.asarray(Wk, f32) * SK),
        "wg8": to8(np.asarray(Wg, f32) * SG),
        "wv": np.ascontiguousarray(np.asarray(Wv, f32).astype(BF)),
        "wo": np.ascontiguousarray(np.asarray(Wo, f32).astype(BF)),
        "bor": np.ascontiguousarray(np.asarray(bo, f32).reshape(CC, 128).T),
        "bgr": np.ascontiguousarray(np.asarray(bg, f32).reshape(CC, 128).T),
        "ident": np.ascontiguousarray(np.eye(128, dtype=f32).astype(BF)),
    }
    pair_exp_t = {}
    bp = np.asarray(bias_pair, f32)[0]  # [H, N, N] (h, q, k)
    for qh in (0, 1):
        sl = bp[:, qh * R:(qh + 1) * R, :]          # [H, R(q), N(k)]
        pair_exp_t[qh] = np.ascontiguousarray(
            np.exp(sl).transpose(0, 2, 1).astype(BF))  # [H, N(k), R(q)]

    in_maps = []
    for i in range(8):
        b, qh = i // 2, i % 2
        m = dict(shared)
        m["qx8"] = to8(np.asarray(q_x[b, qh * R:(qh + 1) * R, :], f32).T)
        m["kv8"] = to8(np.asarray(kv_x[b], f32).T)
        m["kvt"] = np.ascontiguousarray(np.asarray(kv_x[b], f32).T.astype(BF))
        m["maskb"] = np.ascontiguousarray(
            np.asarray(bias_mask[b, 0, 0], f32).reshape(KC, 128).T)
        m["pairt"] = pair_exp_t[qh]
        in_maps.append(m)
    return in_maps


def assemble(results):
    out = np.empty((B, N, C), np.float32)
    for i, r in enumerate(results):
        b, qh = i // 2, i % 2
        out[b, qh * R:(qh + 1) * R, :] = np.asarray(r["out"], np.float32).T
    return out


def kernel(q_x, kv_x, bias_mask, bias_pair, Wq, Wk, Wv, Wg, bg, Wo, bo):
    nc = build()
    in_maps = prep_in_maps(q_x, kv_x, bias_mask, bias_pair,
                           Wq, Wk, Wv, Wg, bg, Wo, bo)
    res = run_bass_kernel_spmd(nc, in_maps, core_ids=list(range(8)))
    return assemble(res.results)


if __name__ == "__main__":
    nc = build()
    print("build OK")


# revision 89
# speedup vs baseline: 1.0081x; 1.0081x over previous
"""Trainium2 Bass kernel for AlphaFold-style gated attention.

Reference math (B=4, N=1024, C=512, H=8, CH=64):
    q = (q_x @ Wq) / 8 ; k = kv_x @ Wk ; v = kv_x @ Wv
    s = q k^T + bias_mask[b,k] + bias_pair[h,q,k]
    a = softmax_k(s) ; o = a @ v
    g = sigmoid(q_x @ Wg + bg)
    out = (o*g) @ Wo + bo

Sharding: 8 cores = (batch b in 0..3) x (q-half qh in 0..1). Zero collectives.

v2 design notes:
  - q/k/g projections run in fp8e4 with DoubleRow perf mode (2
    contraction tiles per instruction, 0.5 cycles/row). Host scales
    weights (x8/x8/x8 after inv_sqrt) into fp8 range; the exp/sigmoid
    activations undo the scale via their constant `scale` operand.
    Scores stay bf16 so q/k carry only one fp8 quantization layer
    (rel err ~1.6e-2 vs the 2e-2 gate).
  - bias_mask folded as the exp() per-partition bias (mask[k] with k on
    partitions) - no em-scaled v needed; the softmax denominator comes
    from a constant-1.0 65th v column.
  - exp(bias_pair) multiplies exp(scores) on DVE/Pool (bf16, 2x mode).
  - AV runs q-on-partitions: one [128q, 65] 8-matmul chain per
    (q-chunk, head) - free dim 65 instead of 512 halves the AV cost -
    and normalization becomes a per-partition reciprocal +
    tensor_scalar multiply, killing the 1/d broadcast matmuls of v1.
  - o/d transposed back to [feat, q] via 16 PE transposes; the gate
    multiply is fused into the transpose's PSUM->SBUF copy.
  - Schedule: the ACT exp stream (32x ~1us) is the pacer. Scores
    trickle through 2-3 PSUM slots at exp pace; projections and the
    previous pair's AV groups are sliced into the stall windows at kc
    granularity. Pair tiles triple-buffer so all four DMA back-to-back;
    mults for pairs 2/3 ride inline behind their exps. The out
    projection opens accumulation chains early (ci 0..2 pre-run), and
    the four output stores dispatch from alternating sequencers.
"""

import sys

import numpy as np

if "/opt/trn_rl_repo" not in sys.path:
    sys.path.insert(0, "/opt/trn_rl_repo")

import ml_dtypes

import concourse.bass as bass
import concourse.tile as tile
from concourse import bacc, mybir
from concourse.bass_utils import run_bass_kernel_spmd

B, N, C, H, CH = 4, 1024, 512, 8, 64
R = 512          # q rows per core
KC = N // 128    # 8 k chunks of 128
CC = C // 128    # 4 feature chunks of 128
F32 = mybir.dt.float32
BF16 = mybir.dt.bfloat16
FP8 = mybir.dt.float8e4
BF = ml_dtypes.bfloat16
F8 = ml_dtypes.float8_e4m3fn

SQ = 64.0        # wq8 = Wq * inv_sqrt * SQ
SK = 8.0         # wk8 = Wk * SK
SG = 8.0         # wg8 = Wg * SG
SEXP = 1.0 / (SQ * SK)


def build(finalize=True):
    nc = bacc.Bacc("TRN2", target_bir_lowering=False, debug=False)

    qx8 = nc.dram_tensor("qx8", [C, R], FP8, kind="ExternalInput").ap()
    bgr = nc.dram_tensor("bgr", [128, CC], F32, kind="ExternalInput").ap()
    kv8 = nc.dram_tensor("kv8", [C, N], FP8, kind="ExternalInput").ap()
    kvt = nc.dram_tensor("kvt", [C, N], BF16, kind="ExternalInput").ap()
    wq8 = nc.dram_tensor("wq8", [C, C], FP8, kind="ExternalInput").ap()
    wk8 = nc.dram_tensor("wk8", [C, C], FP8, kind="ExternalInput").ap()
    wg8 = nc.dram_tensor("wg8", [C, C], FP8, kind="ExternalInput").ap()
    wv = nc.dram_tensor("wv", [C, C], BF16, kind="ExternalInput").ap()
    wo = nc.dram_tensor("wo", [C, C], BF16, kind="ExternalInput").ap()
    pairt = nc.dram_tensor("pairt", [H, N, R], BF16, kind="ExternalInput").ap()
    maskb = nc.dram_tensor("maskb", [128, KC], F32, kind="ExternalInput").ap()
    bor = nc.dram_tensor("bor", [128, CC], F32, kind="ExternalInput").ap()
    ident = nc.dram_tensor("ident", [128, 128], BF16, kind="ExternalInput").ap()
    out = nc.dram_tensor("out", [C, R], BF16, kind="ExternalOutput").ap()

    with tile.TileContext(nc) as tc:
        _body(tc, qx8, kv8, kvt, wq8, wk8, wg8, wv, wo, pairt, maskb, bor,
              bgr, ident, out)
    if finalize:
        nc.finalize()
    return nc


def _body(tc, qx8, kv8, kvt, wq8, wk8, wg8, wv, wo, pairt, maskb, bor,
          bgr, ident, out):
    nc = tc.nc
    Exp = mybir.ActivationFunctionType.Exp
    Sigmoid = mybir.ActivationFunctionType.Sigmoid
    Ident = mybir.ActivationFunctionType.Identity

    with (
        tc.tile_pool(name="keep", bufs=1) as keep,
        tc.tile_pool(name="sb", bufs=8) as sb,
        tc.tile_pool(name="pairp", bufs=3) as pairp,
        tc.tile_pool(name="dp", bufs=2) as dp,
        tc.tile_pool(name="psA", bufs=2, space="PSUM") as psA,
        tc.tile_pool(name="psS", bufs=2, space="PSUM") as psS,
        tc.tile_pool(name="psO", bufs=1, space="PSUM") as psO,
    ):
        # ---- static tiles ----
        maskb_sb = keep.tile([128, KC], F32, tag="maskb")
        bor_sb = keep.tile([128, CC], F32, tag="bor")
        bgr_sb = keep.tile([128, CC], F32, tag="bgr")
        ident_sb = keep.tile([128, 128], BF16, tag="ident")
        qx8_sb = keep.tile([128, CC, R], FP8, tag="qx8")
        kv8_sb = keep.tile([128, CC, N], FP8, tag="kv8")
        kvt_sb = keep.tile([128, CC, N], BF16, tag="kvt")
        w8_sb = {}
        for wname in ("wq8", "wk8", "wg8"):
            w8_sb[wname] = keep.tile([128, CC, C], FP8, tag=wname,
                                     name=f"w_{wname}")
        wv_sb = keep.tile([128, CC, C], BF16, tag="wv")
        wo_sb = keep.tile([128, CC, C], BF16, tag="wo")

        # ---- input DMAs, ordered by first consumer (kproj inputs lead;
        # the small tiles ride later so their HWDGE setup time doesn't
        # delay the first projection) ----
        nc.sync.dma_start(out=kv8_sb, in_=kv8.rearrange("(cc p) n -> p cc n", p=128))
        nc.sync.dma_start(out=w8_sb["wk8"], in_=wk8.rearrange("(cc p) o -> p cc o", p=128))
        nc.sync.dma_start(out=qx8_sb, in_=qx8.rearrange("(cc p) r -> p cc r", p=128))
        nc.sync.dma_start(out=w8_sb["wq8"], in_=wq8.rearrange("(cc p) o -> p cc o", p=128))
        nc.sync.dma_start(out=maskb_sb, in_=maskb)
        nc.sync.dma_start(out=kvt_sb, in_=kvt.rearrange("(cc p) n -> p cc n", p=128))
        nc.sync.dma_start(out=wv_sb, in_=wv.rearrange("(cc p) o -> p cc o", p=128))
        nc.sync.dma_start(out=w8_sb["wg8"], in_=wg8.rearrange("(cc p) o -> p cc o", p=128))
        nc.sync.dma_start(out=bgr_sb, in_=bgr)
        nc.sync.dma_start(out=ident_sb, in_=ident)
        nc.sync.dma_start(out=bor_sb, in_=bor)

        pair_tiles = {}

        def fetch_pair(hp):
            h0 = 2 * hp
            t = pairp.tile([128, KC, 2, R], BF16, tag="pair",
                           name=f"pairt_{hp}")
            for hi in range(2):
                nc.sync.dma_start(
                    out=t[:, :, hi, :],
                    in_=pairt[h0 + hi].rearrange("(kc p) r -> p kc r", p=128))
            pair_tiles[hp] = t

        fetch_pair(0)
        fetch_pair(1)

        # ---- activation / projection result tiles ----
        # qT/kT bf16 [feat, q/n], head pair hp on partition halves 0:64 /
        # 64:128 of feature chunk hp (v1 layout; scores stay bf16 so q/k
        # carry only the projection-input fp8 quantization, not a second
        # storage quantization - keeps rel err at ~1.3e-2).
        qT = keep.tile([128, CC, R], BF16, tag="qT")
        kT = keep.tile([128, CC, N], BF16, tag="kT")
        gT = keep.tile([128, CC, R], BF16, tag="gT")   # [feat, q] gate
        vS = keep.tile([128, KC, H, 65], BF16, tag="vS")
        xg_q = keep.tile([128, 4, C], BF16, tag="xg_q")   # [q(qc), feat]
        xgT = keep.tile([128, CC, R], BF16, tag="xgT")    # [feat, q]

        # ones column for the softmax denominator: vS[:, kc, h, 64] = 1
        nc.vector.memset(vS[:, :, :, 64:65], 1.0)

        # ---- projection emitters (fp8 DoubleRow for k/q/g, bf16 for v) ----
        def emit_kproj(ci, nh):
            ps = psA.tile([128, R], F32, tag="ps", name=f"k_{ci}_{nh}")
            for c2 in range(2):
                nc.tensor.matmul(
                    ps,
                    w8_sb["wk8"][:, 2 * c2:2 * c2 + 2,
                                 ci * 128:(ci + 1) * 128],
                    kv8_sb[:, 2 * c2:2 * c2 + 2, nh * 512:(nh + 1) * 512],
                    start=(c2 == 0), stop=(c2 == 1),
                    perf_mode=mybir.MatmulPerfMode.DoubleRow)
            nc.vector.tensor_copy(kT[:, ci, nh * 512:(nh + 1) * 512], ps)

        def emit_qproj(ci, on_act):
            ps = psA.tile([128, R], F32, tag="ps", name=f"q_{ci}")
            for c2 in range(2):
                nc.tensor.matmul(
                    ps,
                    w8_sb["wq8"][:, 2 * c2:2 * c2 + 2,
                                 ci * 128:(ci + 1) * 128],
                    qx8_sb[:, 2 * c2:2 * c2 + 2, :],
                    start=(c2 == 0), stop=(c2 == 1),
                    perf_mode=mybir.MatmulPerfMode.DoubleRow)
            if on_act:
                nc.scalar.copy(qT[:, ci, :], ps)
            else:
                nc.vector.tensor_copy(qT[:, ci, :], ps)

        g_ps = {}

        def emit_gproj_mm(cc):
            ps = psA.tile([128, R], F32, tag="ps", name=f"g_{cc}")
            for c2 in range(2):
                nc.tensor.matmul(
                    ps,
                    w8_sb["wg8"][:, 2 * c2:2 * c2 + 2,
                                 cc * 128:(cc + 1) * 128],
                    qx8_sb[:, 2 * c2:2 * c2 + 2, :],
                    start=(c2 == 0), stop=(c2 == 1),
                    perf_mode=mybir.MatmulPerfMode.DoubleRow)
            g_ps[cc] = ps

        def emit_sigmoid(cc):
            # g = sigmoid(psum/SG + bg); emitted into the ACT stream at a
            # point where it doesn't delay the exp chain start.
            nc.scalar.activation(gT[:, cc, :], g_ps.pop(cc), Sigmoid,
                                 bias=bgr_sb[:, cc:cc + 1], scale=1.0 / SG)

        def emit_vproj(kc):
            ps = psA.tile([128, R], F32, tag="ps", name=f"v_{kc}")
            for ci in range(CC):
                nc.tensor.matmul(
                    ps, kvt_sb[:, ci, kc * 128:(kc + 1) * 128],
                    wv_sb[:, ci, :], start=(ci == 0), stop=(ci == CC - 1))
            nc.vector.tensor_copy(vS[:, kc, :, 0:64], ps)

        # ---- attention over head pairs ----
        # scores(hp) -> exp -> pair-mult stream one kc at a time (a_t
        # buffered for the whole hp); AV then runs one (qc, head) group at
        # a time, each an 8-matmul contraction chain into its own PSUM
        # bank ([128q, 65] padded - one accumulation group per 2KB zero
        # region). Normalize per-group with a reciprocal + tensor-scalar
        # multiply, then transpose that head-pair's x columns with the
        # gate multiply fused into the PSUM->SBUF copy.

        def emit_score_exp(hp, kc):
            # hp0 borrows the (idle until hp1) AV pool as a third score
            # slot so its scores drain ahead of the sliced projections
            if hp == 0 and kc % 3 == 2:
                st = psO.tile([128, 2, R], F32, tag="ov",
                              name=f"st_{hp}_{kc}")
            else:
                st = psS.tile([128, 2, R], F32, tag="st")
            for hi in range(2):
                p0 = 64 * hi
                nc.tensor.matmul(
                    st[:, hi, :],
                    kT[p0:p0 + 64, hp, kc * 128:(kc + 1) * 128],
                    qT[p0:p0 + 64, hp, :],
                    start=True, stop=True)
            # e deep-buffered so the exp stream never waits on the
            # pair-mults (which chase the pairt DMA)
            e = sb.tile([128, 2, R], BF16, tag="e", bufs=16,
                        name=f"e_{hp}_{kc}")
            nc.scalar.activation(e, st, Exp, bias=maskb_sb[:, kc:kc + 1],
                                 scale=SEXP)
            return e

        def emit_mults(hp, es, pool_kcs=(2, 5)):
            pairt_h = pair_tiles[hp]
            ats = []
            for kc in range(KC):
                a_t = sb.tile([128, 2, R], BF16, tag="at", bufs=18,
                              name=f"a_{hp}_{kc}")
                eng = nc.gpsimd if kc in pool_kcs else nc.vector
                eng.tensor_mul(a_t, es[kc], pairt_h[:, kc, :, :])
                ats.append(a_t)
            return ats

        def emit_av_group(hp, ats, qc, last_hp=False):
            # one [128q, 65] accumulation chain per (qc, head); the two
            # heads of the pair share a 2-bank tile (one group per bank)
            # so the reciprocal batches over both denominator columns.
            # The last pair alternates with the score pool's freed banks
            # so its AV can triple-buffer at the tail.
            h0 = 2 * hp
            if last_hp and qc % 2 == 0:
                ov = psS.tile([128, 2, 65], F32, tag="st",
                              padded_shape=[128, 2, 512],
                              name=f"ov_{hp}_{qc}")
            else:
                ov = psO.tile([128, 2, 65], F32, tag="ov",
                              padded_shape=[128, 2, 512],
                              name=f"ov_{hp}_{qc}")
            for hi in range(2):
                for kc in range(KC):
                    nc.tensor.matmul(
                        ov[:, hi, :],
                        ats[kc][:, hi, qc * 128:(qc + 1) * 128],
                        vS[:, kc, h0 + hi, :],
                        start=(kc == 0), stop=(kc == KC - 1))
            dv = dp.tile([128, 2], F32, tag="dinv",
                         name=f"dinv_{hp}_{qc}")
            nc.vector.reciprocal(dv, ov[:, :, 64])
            for hi in range(2):
                if last_hp and hi == 1:
                    # ACT is idle after the last exp; let it normalize
                    # one head of each pair to shorten the tail
                    nc.scalar.mul(
                        xg_q[:, qc, (h0 + hi) * 64:(h0 + hi + 1) * 64],
                        ov[:, hi, 0:64], dv[:, hi:hi + 1])
                else:
                    nc.vector.tensor_scalar_mul(
                        xg_q[:, qc, (h0 + hi) * 64:(h0 + hi + 1) * 64],
                        ov[:, hi, 0:64], dv[:, hi:hi + 1])

        def emit_transpose(fc, qc):
            # x columns of head pair fc are final; bring them to [feat, q]
            # with the gate fused into the PSUM->SBUF copy. fc3 borrows
            # score-pool slots (psA is held by open out-proj chains then).
            if fc == CC - 1:
                pt = psS.tile([128, 128], BF16, tag="st",
                              padded_shape=[128, 2048],
                              name=f"t_{qc}_{fc}")
            else:
                pt = psA.tile([128, 128], BF16, tag="ps",
                              padded_shape=[128, 1024],
                              name=f"t_{qc}_{fc}")
            nc.tensor.transpose(
                pt, xg_q[:, qc, fc * 128:(fc + 1) * 128], ident_sb)
            nc.vector.tensor_mul(
                xgT[:, fc, qc * 128:(qc + 1) * 128], pt,
                gT[:, fc, qc * 128:(qc + 1) * 128])

        def emit_transposes(fc):
            for qc in range(4):
                emit_transpose(fc, qc)

        # ---- out projection: 4 chains, ci 0..2 pre-run during attention ----
        osb_all = keep.tile([128, CC, R], BF16, tag="osb")
        out_r = out.rearrange("(cc p) r -> cc p r", p=128)
        o_ps = {}

        def emit_oproj_partial(cc, pool, tag, ci_hi):
            if cc not in o_ps:
                if tag == "ps":
                    o_ps[cc] = pool.tile([128, R], F32, tag=tag,
                                         name=f"o_{cc}")
                else:
                    o_ps[cc] = pool.tile([128, R], F32, tag=tag,
                                         padded_shape=[128, 1024],
                                         name=f"o_{cc}")
            ps = o_ps[cc]
            for ci in range(ci_hi[0], ci_hi[1]):
                nc.tensor.matmul(
                    ps, wo_sb[:, ci, cc * 128:(cc + 1) * 128],
                    xgT[:, ci, :], start=(ci == 0), stop=(ci == CC - 1))

        def emit_oproj_finish(cc):
            if cc % 2 == 1:
                nc.scalar.activation(osb_all[:, cc, :], o_ps[cc], Ident,
                                     bias=bor_sb[:, cc:cc + 1])
            else:
                nc.vector.tensor_scalar_add(osb_all[:, cc, :], o_ps[cc],
                                            bor_sb[:, cc:cc + 1])
            # dispatch each chunk's store from its own sequencer so the
            # DGE latencies overlap instead of serializing on one queue
            eng = (nc.sync, nc.scalar, nc.sync, nc.scalar)[cc]
            eng.dma_start(out=out_r[cc], in_=osb_all[:, cc, :])

        # ---- schedule ----
        # Scores trickle through the two st slots at exp pace, so the PE
        # queue behind each score matmul gets a "free" stall window; the
        # remaining projections (iteration 0) and the previous pair's AV
        # groups (iterations 1..3) are sliced into those windows at kc
        # granularity to keep every engine's in-order queue flowing.
        # k00 copy on DVE and q0 on ACT run concurrently; k01 (only
        # needed from kc4) follows on DVE
        emit_kproj(0, 0)
        emit_qproj(0, on_act=True)

        def emit_mult(hp, kc, pool_kcs):
            pairt_h = pair_tiles[hp]
            a_t = sb.tile([128, 2, R], BF16, tag="at", bufs=18,
                          name=f"a_{hp}_{kc}")
            eng = nc.gpsimd if kc in pool_kcs else nc.vector
            eng.tensor_mul(a_t, es[kc], pairt_h[:, kc, :, :])
            return a_t

        ats = None
        for hp in range(H // 2):
            es = []
            cur_ats = []
            # pairs 2/3 are resident before their window starts, so their
            # mults ride inline right behind each exp; pairs 0/1 arrive
            # mid-window and must not stall the DVE queue (batched below)
            inline = hp >= 2
            pool_kcs = (2, 5) if hp < 3 else (2,)
            for kc in range(KC):
                es.append(emit_score_exp(hp, kc))
                if inline:
                    cur_ats.append(emit_mult(hp, kc, pool_kcs))
                if hp == 0:
                    # slice the remaining projections into hp0's windows;
                    # only v0..v3 ride between scores (so kc6/kc7 scores
                    # aren't stuck behind them) - the rest follow kc7.
                    if kc < 3:
                        if kc == 0:
                            emit_kproj(0, 1)
                        emit_kproj(kc + 1, 0)
                        emit_kproj(kc + 1, 1)
                        emit_qproj(kc + 1, on_act=False)
                    elif kc < 7:
                        emit_vproj(kc - 3)
                    else:
                        for cc in range(CC):
                            emit_gproj_mm(cc)
                        for v in range(4, 8):
                            emit_vproj(v)
                else:
                    if hp == 1 and kc == 0:
                        # sigmoids here: the block fills the ACT stream
                        # while hp1's scores catch up
                        for cc in range(CC):
                            emit_sigmoid(cc)
                    if kc % 2 == 1:
                        emit_av_group(hp - 1, ats, (kc - 1) // 2)
            if hp > 0:
                emit_transposes(hp - 1)
            if hp + 2 <= 3:
                fetch_pair(hp + 2)
            if hp == 1:
                nc.sync.dma_start(
                    out=wo_sb, in_=wo.rearrange("(cc p) o -> p cc o", p=128))
            ats = cur_ats if inline else emit_mults(hp, es, pool_kcs)

        # tail: last pair's AV, per-qc transposes, out projection. The
        # first AV group goes before the out-proj partials so its chain
        # steps chase the pair-mult stream; transposes/gates follow each
        # group so the split out-proj sub-chains can chase them.
        emit_av_group(3, ats, 1, last_hp=True)
        emit_oproj_partial(0, psA, "ps", (0, 3))
        emit_oproj_partial(1, psA, "ps", (0, 3))
        for qc in (0, 2, 3):
            emit_av_group(3, ats, qc, last_hp=True)
        emit_transposes(3)

        for cc in range(2):
            emit_oproj_partial(cc, None, None, (3, 4))
            emit_oproj_finish(cc)
        emit_oproj_partial(2, psO, "ov", (0, 4))
        emit_oproj_finish(2)
        emit_oproj_partial(3, psS, "st", (0, 4))
        emit_oproj_finish(3)


def prep_in_maps(q_x, kv_x, bias_mask, bias_pair, Wq, Wk, Wv, Wg, bg, Wo, bo):
    f32 = np.float32
    inv = 1.0 / np.sqrt(np.float32(CH))

    def to8(x):
        return np.ascontiguousarray(np.asarray(x, f32).astype(F8))

    shared = {
        "wq8": to8(np.asarray(Wq, f32) * (inv * SQ)),
        "wk8": to8(np.asarray(Wk, f32) * SK),
        "wg8": to8(np.asarray(Wg, f32) * SG),
        "wv": np.ascontiguousarray(np.asarray(Wv, f32).astype(BF)),
        "wo": np.ascontiguousarray(np.asarray(Wo, f32).astype(BF)),
        "bor": np.ascontiguousarray(np.asarray(bo, f32).reshape(CC, 128).T),
        "bgr": np.ascontiguousarray(np.asarray(bg, f32).reshape(CC, 128).T),
        "ident": np.ascontiguousarray(np.eye(128, dtype=f32).astype(BF)),
    }
    pair_exp_t = {}
    bp = np.asarray(bias_pair, f32)[0]  # [H, N, N] (h, q, k)
    for qh in (0, 1):
        sl = bp[:, qh * R:(qh + 1) * R, :]          # [H, R(q), N(k)]
        pair_exp_t[qh] = np.ascontiguousarray(
            np.exp(sl).transpose(0, 2, 1).astype(BF))  # [H, N(k), R(q)]

    in_maps = []
    for i in range(8):
        b, qh = i // 2, i % 2
        m = dict(shared)
        m["qx8"] = to8(np.asarray(q_x[b, qh * R:(qh + 1) * R, :], f32).T)
        m["kv8"] = to8(np.asarray(kv_x[b], f32).T)
        m["kvt"] = np.ascontiguousarray(np.asarray(kv_x[b], f32).T.astype(BF))
        m["maskb"] = np.ascontiguousarray(
            np.asarray(bias_mask[b, 0, 0], f32).reshape(KC, 128).T)
        m["pairt"] = pair_exp_t[qh]
        in_maps.append(m)
    return in_maps


def assemble(results):
    out = np.empty((B, N, C), np.float32)
    for i, r in enumerate(results):
        b, qh = i // 2, i % 2
        out[b, qh * R:(qh + 1) * R, :] = np.asarray(r["out"], np.float32).T
    return out


def kernel(q_x, kv_x, bias_mask, bias_pair, Wq, Wk, Wv, Wg, bg, Wo, bo):
    nc = build()
    in_maps = prep_in_maps(q_x, kv_x, bias_mask, bias_pair,
                           Wq, Wk, Wv, Wg, bg, Wo, bo)
    res = run_bass_kernel_spmd(nc, in_maps, core_ids=list(range(8)))
    return assemble(res.results)


if __name__ == "__main__":
    nc = build()
    print("build OK")
